# revision 32
# baseline (speedup 1.0000x reference)
"""Trainium2 Bass kernel for nn_NodeInference (2-layer GAT + cosine head).

v2 design (SPMD over 8 cores, dst-node sharding, per-core permuted node
order so each core's own dst shard occupies table rows 0..CAP-1):

  Packed f16 row tables in HBM, 768B stride (ones folded in for SumW):
     row L1 = 2 chunks of 192: [h_c x128 | 1 | as_c | ad_c | pad] (c=head)
     row L2 = [h2 x256 | 1 | as2 ad2 | pad]
  P1  dense-1 (replicated, permuted order): h1aug = x @ W1aug -> h1 table
  Edge phase (per dst block b of 128 dsts; edges grouped by block, chunk 0
  = the appended self-loops in dloc order, then lo-half chunks, hi-half):
    - chunk 0 rows:  direct strided DMA (local shard rows) - no gather
    - other chunks:  dma_gather of 512B rows (the only Q7 descriptor work)
    - a_dst per edge: adval = ST_j^T @ adb on the PE, where ST_j is a
      host-supplied one-hot [128 dst, 128 slot] fp8 and adb is the a_dst
      column of the self chunk (free)
    - w = exp(leakyrelu(a_src + adval, 0.2)) per block (batched ops)
    - S_w[h] = (iota == dstf) * w_h  via batched stride-0-broadcast TTs
    - scatter: bp_h += S_w[h]^T @ h_fp8 ; SumW_h += S_w[h]^T @ ones
    - epilogue: out = bp/SumW + bias (lrelu for layer 1), PE transpose;
      layer 1 also computes h2aug = out1 @ W2aug -> cc_in fp8 rows
  AllGather cc_in -> cc_out (global node order) between the layers.
  Head: headT = g^T @ h2f; cos = (mu.headT) / max(|headT| |mu|, 1e-8).
Host assembles the per-core outT shards.
"""

import sys
from dataclasses import dataclass
from contextlib import ExitStack

if "/opt/trn_rl_repo" not in sys.path:
    sys.path.insert(0, "/opt/trn_rl_repo")

import numpy as np

import concourse.bacc as bacc
import concourse.bass as bass
import concourse.mybir as mybir
import concourse.tile as tile

P = 128
IN = 256          # input feature dim
H1 = 2            # layer-1 heads
HID = 256         # layer-1 output dim (2*128, concat)
OUT = 256         # layer-2 output dim
KH, MD = 8, 128   # cosine head shape
ROWW = 384        # f16 cols per packed table row (768B)
HALF = 32768      # int16 table-half split
AF = mybir.ActivationFunctionType
ALU = mybir.AluOpType
DT = mybir.dt
F8 = DT.float8e4


@dataclass
class CFG:
    N: int
    W: int              # world size
    NBLK: int           # dst blocks (128 dsts) per core
    CPL1: int           # lo-half chunks per block incl self chunk, layer 1
    CPH1: int
    CPL2: int
    CPH2: int
    CPLb1: tuple = ()   # per-block-index lo chunk counts (max over cores)
    CPHb1: tuple = ()
    CPLb2: tuple = ()
    CPHb2: tuple = ()

    @property
    def CAP(self):
        return self.NBLK * P

    @property
    def Npad(self):
        return self.W * self.CAP

    @property
    def NT(self):
        return self.Npad // P

    @property
    def CPB1(self):
        return self.CPL1 + self.CPH1

    @property
    def CPB2(self):
        return self.CPL2 + self.CPH2


def build_program(cfg: CFG):
    nc = bacc.Bacc("TRN2", target_bir_lowering=False, debug=False)
    W, NBLK = cfg.W, cfg.NBLK
    AUG1, AUG2 = IN + 4, IN + 2
    f16, bf16, f32 = DT.float16, DT.bfloat16, DT.float32
    i16 = DT.int16
    CPBX = max(cfg.CPB1, cfg.CPB2)

    with tile.TileContext(nc) as tc, ExitStack() as stack:
        dram = stack.enter_context(
            tc.tile_pool(name="dram", bufs=1, space="DRAM"))

        def din(name, shape, dtype):
            return dram.tile(shape, dtype, kind="ExternalInput", name=name,
                             uniquify=False)

        xTi = din("xTi", [P, cfg.NT, 2, P], f16)
        w1s = din("w1s", [P, 2, AUG1], f16)
        w2s = din("w2s", [P, 2, AUG2], f16)
        gsd = din("gs", [P, 2, KH * P], f16)
        mus = din("mus", [P, KH * KH], f16)       # block-diag mu^T
        ond = din("onesd", [P, KH * KH], f16)     # block-diag ones
        cmu = din("cmu", [KH, 1], f32)
        b1d = din("b1b", [P, HID], f32)
        b2d = din("b2b", [P, OUT], f32)
        iot = din("iotar", [P, CPBX * P], bf16)   # tiled iota 0..127
        idn = din("ident", [P, P], f32)
        is1 = din("isrc1", [P, NBLK * (cfg.CPB1 - 1) * 8], i16)
        is2 = din("isrc2", [P, NBLK * (cfg.CPB2 - 1) * 8], i16)
        df1 = din("dstf1", [P, NBLK * cfg.CPB1], bf16)
        df2 = din("dstf2", [P, NBLK * cfg.CPB2], bf16)
        st1 = din("st1", [P, NBLK * cfg.CPB1 * P], F8)
        st2 = din("st2", [P, NBLK * cfg.CPB2 * P], F8)
        outT = dram.tile([KH, cfg.CAP], f32, kind="ExternalOutput",
                         name="outT", uniquify=False)

        h1t = dram.tile([cfg.Npad, ROWW], f16, name="h1_table")
        cc_in = dram.tile([cfg.CAP, ROWW], f16, name="cc_in")
        # 2-way chunked AllGather: halves are separate Shared tensors so
        # each has a single writer; each also fits int16 gather indexing.
        AGB = [0, min(HALF // P // W, NBLK - 1), NBLK]
        NRA = AGB[1] * P
        NRB = (NBLK - AGB[1]) * P
        cc_aA = dram.tile([W * NRA, ROWW], f16, name="cc_aA",
                          addr_space="Shared" if W > 1 else "Local")
        cc_aB = dram.tile([W * NRB, ROWW], f16, name="cc_aB",
                          addr_space="Shared" if W > 1 else "Local")

        consts = stack.enter_context(tc.tile_pool(name="consts", bufs=1))
        w1_sb = consts.tile([P, 2, AUG1], f16)
        w2_sb = consts.tile([P, 2, AUG2], f16)
        g_sb = consts.tile([P, 2, KH * P], f16)
        mu_sb = consts.tile([P, KH * KH], f16)
        on_sb = consts.tile([P, KH * KH], f16)
        cmu_sb = consts.tile([KH, 1], f32)
        b1_sb = consts.tile([P, HID], f32)
        b2_sb = consts.tile([P, OUT], f32)
        iota_sb = consts.tile([P, CPBX, P], bf16)
        ident_sb = consts.tile([P, P], f32)
        h2fT_sb = consts.tile([P, 2, cfg.CAP], f16)

        for dst, src in [(w1_sb, w1s), (w2_sb, w2s), (g_sb, gsd),
                         (mu_sb, mus), (on_sb, ond), (cmu_sb, cmu),
                         (b1_sb, b1d), (b2_sb, b2d), (iota_sb, iot),
                         (ident_sb, idn)]:
            nc.sync.dma_start(dst[:], src[:])

        # ================= P1: dense layer 1 (replicated, permuted) =========
        # persistent row buffers (2 chunks of 192 each); ones written once
        rowbufs = [consts.tile([P, 4, 2, 192], f16, name=f"rowbuf{i}")
                   for i in range(2)]
        for rb in rowbufs:
            nc.vector.memset(rb[:, :, :, P:P + 1], 1.0)
        XB = 8
        with tc.tile_pool(name="p1x", bufs=3) as p1x, \
             tc.tile_pool(name="p1ps", bufs=2, space="PSUM") as p1ps:
            assert cfg.NT % XB == 0
            for t0 in range(0, cfg.NT, XB):
                xt = p1x.tile([P, XB, 2, P], f16, tag="xt")
                nc.sync.dma_start(xt[:], xTi[:, t0:t0 + XB, :, :])
                for q0 in range(0, XB, 4):
                    # 4 node-tiles share one packed row buffer + one write
                    row = rowbufs[(t0 + q0) // 4 % 2]
                    for q in range(4):
                        ps = p1ps.tile([P, AUG1], f32, tag="ps")
                        for k in range(2):
                            nc.tensor.matmul(ps[:], lhsT=xt[:, q0 + q, k, :],
                                             rhs=w1_sb[:, k, :],
                                             start=(k == 0), stop=(k == 1))
                        nc.scalar.activation(row[:, q, :, 0:P],
                                             ps[:, 0:IN], AF.Copy)
                        nc.vector.tensor_copy(row[:, q, :, P + 1:P + 3],
                                              ps[:, IN:IN + 4])
                    t = t0 + q0
                    wdst = h1t[t * P:(t + 4) * P, :]
                    wdst = wdst.__replace__(
                        ap=[[wdst.ap[0][0], P], [P * ROWW, 4], [1, ROWW]])
                    nc.sync.dma_start(wdst, row[:, :, :, :])

        # ================= edge phases ======================================
        def edge_phase(layer):
            if layer == 1:
                self_tab = h1t
                tab_lo, tab_hi = h1t[0:HALF, :], h1t[HALF:cfg.Npad, :]
                isrc_d, dstf_d, st_d = is1, df1, st1
                CPL, CPH, CPB = cfg.CPL1, cfg.CPH1, cfg.CPB1
                CPLb, CPHb = cfg.CPLb1, cfg.CPHb1
                H = 2
                b_sb, lrelu_out = b1_sb, True
            else:
                self_tab = cc_in
                tab_lo, tab_hi = cc_aA[:, :], cc_aB[:, :]
                isrc_d, dstf_d, st_d = is2, df2, st2
                CPL, CPH, CPB = cfg.CPL2, cfg.CPH2, cfg.CPB2
                CPLb, CPHb = cfg.CPLb2, cfg.CPHb2
                H = 1
                b_sb, lrelu_out = b2_sb, False
            GW = CPB - 1          # gathered chunks per block

            estack = ExitStack()
            pi = estack.enter_context(
                tc.tile_pool(name=f"idx{layer}", bufs=2))
            pg = estack.enter_context(
                tc.tile_pool(name=f"gath{layer}", bufs=2))
            pe_ = estack.enter_context(tc.tile_pool(name=f"ew{layer}", bufs=2))
            psw = estack.enter_context(tc.tile_pool(name=f"sw{layer}", bufs=2))
            pap = estack.enter_context(
                tc.tile_pool(name=f"adp{layer}", bufs=2, space="PSUM"))
            pp = estack.enter_context(
                tc.tile_pool(name=f"bps{layer}", bufs=2, space="PSUM"))
            pt = estack.enter_context(
                tc.tile_pool(name=f"tps{layer}", bufs=2, space="PSUM"))
            po = estack.enter_context(tc.tile_pool(name=f"epi{layer}", bufs=2))
            ph = estack.enter_context(
                tc.tile_pool(name=f"h2ps{layer}", bufs=2, space="PSUM"))
            pgm = estack.enter_context(
                tc.tile_pool(name=f"gtb{layer}", bufs=1))
            gtbufs = [pgm.tile([P, CPB, ROWW], f16, name=f"gtb{layer}_{i}")
                      for i in range(2)]
            for gb in gtbufs:
                nc.vector.memset(gb[:], 0.0)

            for blk in range(NBLK):
                st = pg.tile([P, CPB, P], F8, tag="st")
                dsf = pi.tile([P, CPB], bf16, tag="dsf")
                gt = gtbufs[blk % 2]
                nc.sync.dma_start(st[:], st_d[:, blk * CPB * P:
                                              (blk + 1) * CPB * P])
                nc.sync.dma_start(dsf[:], dstf_d[:, blk * CPB:
                                                 (blk + 1) * CPB])
                isr = pi.tile([P, GW * 8], i16, tag="isr")
                nc.sync.dma_start(isr[:], isrc_d[:, blk * GW * 8:
                                                 (blk + 1) * GW * 8])
                # self chunk: local shard rows, direct strided DMA
                if H == 2:
                    sdst = gt[:, 0, 0:ROWW].__replace__(
                        ap=[gt[:, 0, 0:ROWW].ap[0], [192, 2], [1, 131]])
                    ssrc = self_tab[blk * P:(blk + 1) * P, :]
                    ssrc = ssrc.__replace__(
                        ap=[ssrc.ap[0], [192, 2], [1, 131]])
                    nc.sync.dma_start(sdst, ssrc)
                else:
                    SRW = IN + 3
                    nc.sync.dma_start(gt[:, 0, 0:SRW],
                                      self_tab[blk * P:(blk + 1) * P, 0:SRW])
                # gathered chunks (chunk index 1..CPB-1 -> isr slot c-1);
                # per-block-index extents (max over cores), rest stays dead
                MXC = 8
                cl = CPLb[blk] if CPLb else CPL
                chx = CPL + (CPHb[blk] if CPHb else CPH)
                for c0 in range(1, cl, MXC):
                    c1 = min(c0 + MXC, cl)
                    nc.gpsimd.dma_gather(
                        gt[:, c0:c1, :], tab_lo,
                        isr[:, (c0 - 1) * 8:(c1 - 1) * 8],
                        (c1 - c0) * P, (c1 - c0) * P, ROWW)
                for c0 in range(CPL, chx, MXC):
                    c1 = min(c0 + MXC, chx)
                    nc.gpsimd.dma_gather(
                        gt[:, c0:c1, :], tab_hi,
                        isr[:, (c0 - 1) * 8:(c1 - 1) * 8],
                        (c1 - c0) * P, (c1 - c0) * P, ROWW)

                def gsel(j):
                    return gt[:, j, :]

                # adval[e, h] = ST_j^T @ adb ; adb = a_dst cols of self chunk
                if H == 2:
                    a0 = gt[:, 0, P + 2:P + 3]
                    adb = a0.__replace__(ap=[a0.ap[0], [192, 2], [1, 1]])
                    v0 = gt[:, :, P + 1:P + 2]
                    as_v = v0.__replace__(
                        ap=[v0.ap[0], [ROWW, CPB], [192, 2], [1, 1]])
                else:
                    adb = gt[:, 0, IN + 2:IN + 3]
                    as_v = gt[:, :, IN + 1:IN + 2]
                advalP = pap.tile([P, CPB, H], f32, tag="adval")
                for j in range(CPB):
                    nc.tensor.matmul(advalP[:, j, :], lhsT=st[:, j, :],
                                     rhs=adb, start=True, stop=True)
                # ew = a_src + adval ; w = exp(leakyrelu(ew, .2))
                ew = pe_.tile([P, CPB, H], f32, tag="ew")
                wv = pe_.tile([P, CPB, H], f32, tag="wv")
                nc.vector.tensor_tensor(ew[:], as_v, advalP[:], op=ALU.add)
                nc.vector.tensor_scalar(out=wv[:], in0=ew[:], scalar1=0.2,
                                        scalar2=None, op0=ALU.mult)
                nc.vector.tensor_tensor(wv[:], wv[:], ew[:], op=ALU.max)
                nc.scalar.activation(wv[:], wv[:], AF.Exp)

                # S_w[h] = (iota == dstf)*w_h  (batched, stride-0 bcasts)
                S = psw.tile([P, CPB, P], bf16, tag="S")
                nc.vector.tensor_tensor(
                    S[:], iota_sb[:, 0:CPB, :],
                    dsf[:].broadcast_to([P, CPB, P]),
                    op=ALU.is_equal)
                sw = []
                for h in range(H):
                    swh = psw.tile([P, CPB, P], bf16, tag=f"swh{h}")
                    nc.vector.tensor_tensor(
                        swh[:], S[:],
                        wv[:, :, h:h + 1].broadcast_to([P, CPB, P]),
                        op=ALU.mult)
                    sw.append(swh)

                # scatter: bp_h += S_w[h]^T @ [h_cols | 1]
                # chains strictly sequential: one open accumulation at a time
                FW = P if H == 2 else OUT
                FS = FW + 64
                bp = pp.tile([P, H, FS], f32, tag="bp")
                for h in range(H):
                    for j in range(CPB):
                        rhs = gsel(j)[:, h * 192:h * 192 + FW + 1]
                        nc.tensor.matmul(bp[:, h, 0:FW + 1],
                                         lhsT=sw[h][:, j, :],
                                         rhs=rhs, start=(j == 0),
                                         stop=(j == CPB - 1))

                # ---- block epilogue
                rec = po.tile([P, H], f32, tag="rec")
                nc.vector.tensor_copy(rec[:], bp[:, :, FW])

                nc.vector.reciprocal(rec[:], rec[:])
                ti = po.tile([P, IN], f32, tag="ti")
                for h in range(H):
                    w0 = h * (IN // H)
                    nc.vector.tensor_scalar(
                        out=ti[:, w0:w0 + IN // H], in0=bp[:, h, 0:FW],
                        scalar1=rec[:, h:h + 1], scalar2=None, op0=ALU.mult)
                nc.vector.tensor_tensor(ti[:], ti[:], b_sb[:], op=ALU.add)
                if lrelu_out:
                    tl = po.tile([P, IN], f32, tag="tl")
                    nc.vector.tensor_scalar(out=tl[:], in0=ti[:],
                                            scalar1=0.01, scalar2=None,
                                            op0=ALU.mult)
                    nc.vector.tensor_tensor(ti[:], tl[:], ti[:], op=ALU.max)

                if layer == 1:
                    # h2aug = out1 @ W2aug -> cc_in fp8 rows
                    o1t = po.tile([P, 2, P], f16, tag="o1t")
                    for k in range(2):
                        tp = pt.tile([P, P], f32, tag="tp")
                        nc.tensor.transpose(tp[:], ti[:, k * P:(k + 1) * P],
                                            ident_sb[:])
                        if k == 0:
                            nc.scalar.activation(o1t[:, k, :], tp[:], AF.Copy)
                        else:
                            nc.vector.tensor_copy(o1t[:, k, :], tp[:])
                    hp = ph.tile([P, AUG2], f32, tag="hp")
                    for k in range(2):
                        nc.tensor.matmul(hp[:], lhsT=o1t[:, k, :],
                                         rhs=w2_sb[:, k, :],
                                         start=(k == 0), stop=(k == 1))
                    row2 = po.tile([P, ROWW], f16, tag="row2")
                    nc.scalar.activation(row2[:, 0:OUT], hp[:, 0:OUT],
                                         AF.Copy)
                    nc.vector.memset(row2[:, OUT:OUT + 1], 1.0)
                    nc.vector.tensor_copy(row2[:, OUT + 1:OUT + 3],
                                          hp[:, OUT:OUT + 2])
                    nc.sync.dma_start(
                        cc_in[blk * P:(blk + 1) * P, 0:OUT + 3],
                        row2[:, 0:OUT + 3])
                else:
                    # h2 final -> h2fT slices via PE transpose
                    for k in range(2):
                        tp = pt.tile([P, P], f32, tag="tp")
                        nc.tensor.transpose(tp[:], ti[:, k * P:(k + 1) * P],
                                            ident_sb[:])
                        dsl = h2fT_sb[:, k, blk * P:(blk + 1) * P]
                        if k == 0:
                            nc.scalar.activation(dsl, tp[:], AF.Copy)
                        else:
                            nc.vector.tensor_copy(dsl, tp[:])

                if layer == 1 and (blk + 1) in AGB[1:]:
                    k = AGB.index(blk + 1) - 1
                    r0, r1 = AGB[k] * P, AGB[k + 1] * P
                    cc_t = cc_aA if k == 0 else cc_aB
                    if W > 1:
                        nc.gpsimd.collective_compute(
                            "AllGather", ALU.bypass,
                            replica_groups=[list(range(W))],
                            ins=[cc_in[r0:r1, :]], outs=[cc_t[:]])
                    else:
                        nc.sync.dma_start(cc_t[0:r1 - r0, :],
                                          cc_in[r0:r1, :])

            estack.close()

        # head tile: cosine vs mu over h2fT cols [stp, stp+wdt)
        def head_tile(hpools, stp, wdt):
            hps, hsb, sps, hepi = hpools
            nump = sps.tile([KH, 512], f32, tag="nump")
            nrmp = sps.tile([KH, 512], f32, tag="nrmp")
            for k in range(KH):
                hp = hps.tile([P, 512], f32, tag="hp")
                for f in range(2):
                    nc.tensor.matmul(hp[:, 0:wdt],
                                     lhsT=g_sb[:, f, k * P:(k + 1) * P],
                                     rhs=h2fT_sb[:, f, stp:stp + wdt],
                                     start=(f == 0), stop=(f == 1))
                h16 = hsb.tile([P, 512], f16, tag="h16")
                sq16 = hsb.tile([P, 512], f16, tag="sq16")
                nc.vector.tensor_copy(h16[:, 0:wdt], hp[:, 0:wdt])
                nc.scalar.activation(sq16[:, 0:wdt], hp[:, 0:wdt],
                                     AF.Square)
                nc.tensor.matmul(nump[:, 0:wdt],
                                 lhsT=mu_sb[:, k * KH:(k + 1) * KH],
                                 rhs=h16[:, 0:wdt], start=(k == 0),
                                 stop=(k == KH - 1))
                nc.tensor.matmul(nrmp[:, 0:wdt],
                                 lhsT=on_sb[:, k * KH:(k + 1) * KH],
                                 rhs=sq16[:, 0:wdt], start=(k == 0),
                                 stop=(k == KH - 1))
            sq = hepi.tile([KH, 512], f32, tag="sqr")
            nc.scalar.activation(sq[:, 0:wdt], nrmp[:, 0:wdt], AF.Sqrt)
            nc.vector.tensor_scalar(out=sq[:, 0:wdt], in0=sq[:, 0:wdt],
                                    scalar1=cmu_sb[:, 0:1], scalar2=1e-8,
                                    op0=ALU.mult, op1=ALU.max)
            nc.vector.reciprocal(sq[:, 0:wdt], sq[:, 0:wdt])
            res = hepi.tile([KH, 512], f32, tag="res")
            nc.vector.tensor_tensor(res[:, 0:wdt], nump[:, 0:wdt],
                                    sq[:, 0:wdt], op=ALU.mult)
            nc.sync.dma_start(outT[:, stp:stp + wdt], res[:, 0:wdt])

        edge_phase(1)
        edge_phase(2)
        with tc.tile_pool(name="hps", bufs=2, space="PSUM") as hps, \
             tc.tile_pool(name="hsb", bufs=3) as hsb, \
             tc.tile_pool(name="sps", bufs=2, space="PSUM") as sps, \
             tc.tile_pool(name="hepi", bufs=2) as hepi:
            hpools = (hps, hsb, sps, hepi)
            for ht in range((cfg.CAP + 511) // 512):
                head_tile(hpools, ht * 512, min(512, cfg.CAP - ht * 512))

    nc.compile()
    return nc


# ======================= host-side preparation ==============================

def _wrap16(flat):
    """idx flat [n] -> wrapped int16 [128, n//16]; pos i -> (i%16, i//16),
    replicated across the 8 Q7-core stripes."""
    n = len(flat)
    out = np.zeros((P, max(n // 16, 1)), np.int16)
    if n == 0:
        return out
    cols = np.arange(n) // 16
    rows = np.arange(n) % 16
    for r in range(8):
        out[r * 16 + rows, cols] = flat
    return out


def prep_host(x, edge_index, W1, a_src1, a_dst1, b1, W2, a_src2, a_dst2, b2,
              g, mu, world=8):
    import ml_dtypes
    x = np.asarray(x, np.float32)
    N = x.shape[0]
    NBLK = int(np.ceil(N / world / P))
    CAP = NBLK * P
    Npad = world * CAP

    # non-self edges in global slot ids
    src_e = np.asarray(edge_index[0]).astype(np.int64)
    dst_e = np.asarray(edge_index[1]).astype(np.int64)
    core_e = dst_e // CAP
    blk_e = (dst_e - core_e * CAP) // P

    gkey = core_e * NBLK + blk_e
    gorder = np.argsort(gkey, kind="stable")
    src_s, dst_s, gkey_s = src_e[gorder], dst_e[gorder], gkey[gorder]
    starts = np.concatenate(
        [[0], np.cumsum(np.bincount(gkey_s, minlength=world * NBLK))])

    # per-core node permutation: own shard first (over padded slot space)
    perms, invs = [], []
    for c in range(world):
        lo, hi = c * CAP, (c + 1) * CAP
        perm = np.concatenate([np.arange(lo, hi), np.arange(0, lo),
                               np.arange(hi, Npad)])
        inv = np.empty(Npad, np.int64)
        inv[perm] = np.arange(Npad)
        perms.append(perm)
        invs.append(inv)

    # layer-2 table remap: 2-way chunked AllGather (A half / B half)
    AGB = [0, min(HALF // P // world, NBLK - 1), NBLK]
    NRA = AGB[1] * P
    remap = np.zeros(Npad, np.int64)   # position within its half-table
    inA = np.zeros(Npad, bool)
    for c in range(world):
        lr = np.arange(CAP)
        gl = c * CAP + lr
        a = lr < NRA
        remap[gl[a]] = c * NRA + lr[a]
        inA[gl[a]] = True
        remap[gl[~a]] = c * (CAP - NRA) + (lr[~a] - NRA)
    # per (core, block, layer): split non-self edges by table half
    ed = {}
    CPLb1 = np.full(NBLK, 2, np.int64)
    CPHb1 = np.zeros(NBLK, np.int64)
    CPLb2 = np.full(NBLK, 2, np.int64)
    CPHb2 = np.zeros(NBLK, np.int64)
    for c in range(world):
        inv = invs[c]
        for b in range(NBLK):
            gid = c * NBLK + b
            es = src_s[starts[gid]:starts[gid + 1]]
            eds = dst_s[starts[gid]:starts[gid + 1]]
            dloc = (eds - c * CAP - b * P).astype(np.int64)
            l1 = inv[es]
            l2 = remap[es]
            lo1 = l1 < HALF
            lo2 = inA[es]
            ed[(c, b)] = (l1, lo1, l2, lo2, dloc)
            CPLb1[b] = max(CPLb1[b], 1 + int(np.ceil(lo1.sum() / P)))
            CPHb1[b] = max(CPHb1[b], int(np.ceil((~lo1).sum() / P)))
            CPLb2[b] = max(CPLb2[b], 1 + int(np.ceil(lo2.sum() / P)))
            CPHb2[b] = max(CPHb2[b], int(np.ceil((~lo2).sum() / P)))

    cfg = CFG(N=N, W=world, NBLK=NBLK,
              CPL1=int(CPLb1.max()), CPH1=int(CPHb1.max()),
              CPL2=int(CPLb2.max()), CPH2=int(CPHb2.max()),
              CPLb1=tuple(int(x) for x in CPLb1),
              CPHb1=tuple(int(x) for x in CPHb1),
              CPLb2=tuple(int(x) for x in CPLb2),
              CPHb2=tuple(int(x) for x in CPHb2))

    def build_layer(c, lnum):
        CPL = cfg.CPL1 if lnum == 1 else cfg.CPL2
        CPB = cfg.CPB1 if lnum == 1 else cfg.CPB2
        GW = CPB - 1
        isrc = np.zeros((P, NBLK * GW * 8), np.int16)
        dstf = np.full((NBLK * CPB, P), -1.0, np.float32)
        stm = np.zeros((NBLK * CPB, P, P), np.uint8)  # [chunk, d, slot]
        one_f8 = np.float32(1).astype(ml_dtypes.float8_e4m3).view(np.uint8)
        iden = np.zeros((P, P), np.uint8)
        iden[np.arange(P), np.arange(P)] = one_f8
        for b in range(NBLK):
            l1, lo1, l2, lo2, dloc = ed[(c, b)]
            ids, lo = (l1, lo1) if lnum == 1 else (l2, lo2)
            # slot -> table idx / dloc for gathered chunks (0 = chunk 1)
            fl = np.zeros(GW * P, np.int64)
            fd = np.full(GW * P, -1, np.int64)
            ilo = np.where(lo)[0]
            ihi = np.where(~lo)[0]
            fl[:len(ilo)] = ids[ilo]
            fd[:len(ilo)] = dloc[ilo]
            hb = (CPL - 1) * P
            fl[hb:hb + len(ihi)] = ids[ihi] - (HALF if lnum == 1 else 0)
            fd[hb:hb + len(ihi)] = dloc[ihi]
            isrc[:, b * GW * 8:(b + 1) * GW * 8] = _wrap16(fl)
            # dstf: chunk 0 = iota (self), others from fd
            dstf[b * CPB] = np.arange(P)
            dstf[b * CPB + 1:(b + 1) * CPB] = fd.reshape(GW, P)
            # one-hot ST [d, slot] per chunk
            stm[b * CPB] = iden
            fd2 = fd.reshape(GW, P)
            cc, ss = np.where(fd2 >= 0)
            stm[b * CPB + 1 + cc, fd2[cc, ss], ss] = one_f8
        dstfw = np.ascontiguousarray(dstf.T).astype(ml_dtypes.bfloat16)
        stw = np.ascontiguousarray(
            stm.transpose(1, 0, 2).reshape(P, NBLK * CPB * P))
        return isrc, dstfw, stw.view(ml_dtypes.float8_e4m3)

    # weights
    W1 = np.asarray(W1, np.float32)
    W2 = np.asarray(W2, np.float32)
    W1r = W1.reshape(H1, MD, IN)
    Ps1 = np.einsum("hdi,hd->ih", W1r, np.asarray(a_src1, np.float32))
    Pd1 = np.einsum("hdi,hd->ih", W1r, np.asarray(a_dst1, np.float32))
    W1aug = np.concatenate([W1.T, Ps1[:, 0:1], Pd1[:, 0:1],
                            Ps1[:, 1:2], Pd1[:, 1:2]], axis=1)
    Ps2 = W2.T @ np.asarray(a_src2, np.float32)[0][:, None]
    Pd2 = W2.T @ np.asarray(a_dst2, np.float32)[0][:, None]
    W2aug = np.concatenate([W2.T, Ps2, Pd2], axis=1)
    AUG1, AUG2 = IN + 4, IN + 2
    w1s = W1aug.reshape(2, P, AUG1).transpose(1, 0, 2).astype(np.float16)
    w2s = W2aug.reshape(2, P, AUG2).transpose(1, 0, 2).astype(np.float16)

    gm = np.asarray(g, np.float32)
    gsd = gm.reshape(2, P, KH * P).transpose(1, 0, 2).astype(np.float16)
    mu = np.asarray(mu, np.float32)
    mus = np.zeros((P, KH * KH), np.float16)
    onesd = np.zeros((P, KH * KH), np.float16)
    for k in range(KH):
        mus[:, k * KH + k] = mu[k, :]
        onesd[:, k * KH + k] = 1.0
    cmu = np.linalg.norm(mu, axis=1)[:, None].astype(np.float32)
    b1b = np.broadcast_to(np.asarray(b1, np.float32), (P, HID)).copy()
    b2b = np.broadcast_to(np.asarray(b2, np.float32), (P, OUT)).copy()
    CPBX = max(cfg.CPB1, cfg.CPB2)
    iotar = np.broadcast_to(
        np.tile(np.arange(P, dtype=np.float32), CPBX),
        (P, CPBX * P)).astype(ml_dtypes.bfloat16)
    ident = np.eye(P, dtype=np.float32)

    shared = dict(w1s=w1s, w2s=w2s, gs=gsd, mus=mus, onesd=onesd, cmu=cmu,
                  b1b=b1b, b2b=b2b, iotar=iotar, ident=ident)
    in_maps = []
    for c in range(world):
        xs = np.zeros((Npad, IN), np.float32)
        valid = perms[c] < N
        xs[valid] = x[perms[c][valid]]
        xTi = xs.reshape(cfg.NT, P, 2, P).transpose(3, 0, 2, 1).astype(
            np.float16)
        i1, d1, s1 = build_layer(c, 1)
        i2, d2, s2 = build_layer(c, 2)
        m = dict(shared)
        m.update(xTi=xTi, isrc1=i1, dstf1=d1, st1=s1,
                 isrc2=i2, dstf2=d2, st2=s2)
        in_maps.append(m)
    return cfg, in_maps


_CACHE = {}


def kernel(**inputs):
    world = 8
    cfg, in_maps = prep_host(world=world, **inputs)
    key = (cfg.N, cfg.W, cfg.CPB1, cfg.CPB2,
           cfg.CPLb1, cfg.CPHb1, cfg.CPLb2, cfg.CPHb2)
    if key not in _CACHE:
        _CACHE[key] = build_program(cfg)
    nc = _CACHE[key]

    from concourse.bass_utils import run_bass_kernel_spmd
    res = run_bass_kernel_spmd(nc, in_maps, core_ids=list(range(world)))
    outs = res.results
    N, CAP = cfg.N, cfg.CAP
    full = np.zeros((N, KH), np.float32)
    for c in range(world):
        o = outs[c]["outT"]
        n = min(CAP, N - c * CAP)
        if n > 0:
            full[c * CAP:c * CAP + n, :] = o[:, :n].T
    return full
